# revision 1
# baseline (speedup 1.0000x reference)
"""Trainium2 Bass kernel for ContinuousWaveletLayer (CWT energy).

Reference computation:
  bank = Morlet wavelet bank [32 scales, Lmax=256] (static)
  coef[b,s,t] = 'same' conv of x[b,:] (len 8192) with bank[s,:]
  out[b,s]    = mean_t(coef^2) * softmax(scale_weights)[s]

Device strategy (8 NeuronCores, scale-parallel, 4 scales/core):
  The conv is phrased as Toeplitz matmuls on the tensor engine. With x
  zero-padded (128 left / 128 right) and viewed time-major in blocks of
  128, the output block B (128 time steps) for scale s is

      coef_B[to, b] = sum_{D=0..2} G[s,D].T @ Xblk[B+D]          (K=128)

  where G[s,D][a,to] = g_s[128*D + a - to] (g_s = reversed bank row,
  zero outside [0,256)) and Xblk[A][a,b] = xpad[128*A + a, b].
  x is stored in SBUF as [a=128 partitions, (A,b) free], so the rhs for
  (D, 4-block group) is just a contiguous 512-wide free-dim slice; the
  3 D-matmuls accumulate in PSUM.  Squares are computed on the scalar /
  vector engines (alternating) and accumulated in SBUF; the final
  partition reduction is a ones-vector matmul.  Host applies the final
  1/N and softmax scaling on the gathered [32,128] sums (O(4K) flops).
"""

import os
import sys
from contextlib import ExitStack

import numpy as np

sys.path.insert(0, "/opt/trn_rl_repo")

import concourse.bass as bass
import concourse.mybir as mybir
from concourse import tile
from concourse.bass_utils import run_bass_kernel_spmd
from concourse.vector_clock import ScopedClock


def _drain_and_barrier_single_wait(self, tick_clock, wait_clock):
    """TileContext._drain_and_barrier, but the kernel-tail drain's
    global-clock waits are spread over a chain of single-wait drains —
    the walrus build here allows only one sync wait per instruction."""
    drain_inst = self.nc.sync.drain()
    wait_clock.add_sem_waits(
        drain_inst.ins, ScopedClock({None: tick_clock.global_clock})
    )
    si = drain_inst.ins.sync_info
    waits = list(si.on_wait)
    if len(waits) > 1:
        si.on_wait = [waits[0]]
        sems = {h.name: h for h in self.sems.allocated().values()}
        for w in waits[1:]:
            d2 = self.nc.sync.drain()
            d2.wait_op(sems[w.ant_name], w.wait_value, "sem-ge")
    self.nc.all_engine_barrier()
    assert self.sems is not None
    popped = self.nc._tile_sem_poison_stack.pop()
    assert popped is self._sem_poison
    self.nc.clear_and_free_semaphores(list(self.sems.allocated().values()))
    self.nc.all_engine_barrier()


tile.TileContext._drain_and_barrier = _drain_and_barrier_single_wait

N_CORES = 8
S_TOTAL = 32          # number of scales
S_PER = 4             # scales per core
P = 128               # partition / block size
NT = 8192             # time samples
LMAX = 256            # padded kernel length
NBLK = 66             # input blocks: (128 + 8192 + 128) / 128
NOUT = 64             # output blocks: 8192 / 128
NGRP = 16             # groups of 4 output blocks (N=512 matmuls)
F32 = mybir.dt.float32
BF16 = mybir.dt.bfloat16

LAST_RESULTS = None   # BassKernelResults of the most recent run (for test.py)


def _morlet_kernel_bank(n_scales: int, n: int) -> np.ndarray:
    Lmax = min(8 * n_scales, n)
    bank = np.zeros((n_scales, Lmax), dtype=np.float32)
    for i, s in enumerate(range(1, n_scales + 1)):
        L = min(8 * s, n)
        t = np.linspace(-4.0 * s, 4.0 * s, L)
        w = np.exp(-t**2 / (2.0 * s**2)) * np.cos(5.0 * t / s)
        w = w / np.sqrt(s)
        off = (Lmax - 1) // 2 - (L - 1) // 2
        bank[i, off : off + L] = w.astype(np.float32)
    return bank


def _toeplitz_weights() -> np.ndarray:
    """G[s, D][a, to] = g_s[128*D + a - to], zero outside support."""
    bank = _morlet_kernel_bank(S_TOTAL, NT)          # [32, 256]
    g = bank[:, ::-1].copy()                         # reversed rows
    a = np.arange(P)[:, None]
    to = np.arange(P)[None, :]
    G = np.zeros((S_TOTAL, 3, P, P), dtype=np.float32)
    for D in range(3):
        d = 128 * D + a - to
        valid = (d >= 0) & (d < LMAX)
        dc = np.clip(d, 0, LMAX - 1)
        for s in range(S_TOTAL):
            G[s, D] = np.where(valid, g[s][dc], 0.0)
    return G


GCOLS = S_PER * 3 * P          # 1536 weight columns
XCOLS = NBLK * P               # 8448 x columns


def _build_nc() -> bass.Bass:
    nc = bass.Bass()
    # combined input, one DMA → one semaphore lane for every matmul dep:
    #   xg[:, :GCOLS]    = per-core Toeplitz weights (G[s,D,a,to])
    #   xg[:, GCOLS:-1]  = x time-major: xpad[128*A + a, b]
    #   xg[:, -1]        = ones column (partition reducer)
    xg = nc.dram_tensor("xg", [P, GCOLS + XCOLS + 1], BF16, kind="ExternalInput")
    # per-core partial energies, un-folded: outp[s, (Bsub, b)]; the host
    # sums the 4 column groups (keeps DVE out of the kernel → fewer
    # semaphore procs for the tail drain)
    outp = nc.dram_tensor("outp", [1, S_PER * 512], F32, kind="ExternalOutput")

    with tile.TileContext(nc) as tc, ExitStack() as ctx:
        xpool = ctx.enter_context(tc.tile_pool(name="x", bufs=1))
        # one sq buffer per (ng, s): no slot reuse → no WAR-induced second
        # wait on the ACT squares (walrus allows 1 sync wait/instruction)
        sqpool = ctx.enter_context(tc.tile_pool(name="sq", bufs=NGRP * S_PER))
        # fp32 PSUM-evict scratch for the DVE square path; slot WARs are
        # DVE-vs-DVE (same engine) so reuse costs no extra waits
        cppool = ctx.enter_context(tc.tile_pool(name="cp", bufs=4))
        rowpool = ctx.enter_context(tc.tile_pool(name="row", bufs=4))
        pspool = ctx.enter_context(tc.tile_pool(name="ps", bufs=4, space="PSUM"))
        psepool = ctx.enter_context(tc.tile_pool(name="pse", bufs=1, space="PSUM"))

        xgsb = xpool.tile([P, GCOLS + XCOLS + 1], BF16)
        # one DMA: a single InstDMACopy fans out across all 16 SDMA engines
        nc.sync.dma_start(out=xgsb[:, :], in_=xg[:, :])
        onesb = xgsb[:, GCOLS + XCOLS : GCOLS + XCOLS + 1]

        # per-scale PSUM energy accumulators [1, (Bsub, b)]
        pes = [
            psepool.tile([1, 512], F32, tag=f"pe{s}", name=f"pe{s}")
            for s in range(S_PER)
        ]

        # main conv loop; all cross-engine deps are 1-wait:
        #   conv matmul:  DMA sem (once) / evict-engine sem (bank recycle)
        #   evict+square: PE sem (ACT path) or PE sem + DVE-self (DVE path)
        #   reduce matmul (PE, accumulates into pes[s]): ACT/DVE sem
        for ng in range(NGRP):
            for s in range(S_PER):
                pt = pspool.tile([P, 512], F32)
                for D in range(3):
                    gc = (s * 3 + D) * P
                    xc = GCOLS + (ng * 4 + D) * P
                    lhsT = xgsb[:, gc : gc + P]
                    rhs = xgsb[:, xc : xc + 4 * P]
                    nc.tensor.matmul(
                        pt[:, :], lhsT, rhs, start=(D == 0), stop=(D == 2)
                    )
                sq = sqpool.tile([P, 512], BF16)
                if (ng * S_PER + s) % 2 == 0:
                    # ACT path: square+cast straight out of PSUM
                    nc.scalar.square(sq[:, :], pt[:, :])
                else:
                    # DVE path: fp32 copy out of PSUM, then square+cast
                    cp = cppool.tile([P, 512], F32)
                    nc.vector.tensor_copy(cp[:, :], pt[:, :])
                    nc.vector.tensor_mul(sq[:, :], cp[:, :], cp[:, :])
                nc.tensor.matmul(
                    pes[s][:, :],
                    onesb,
                    sq[:, :],
                    start=(ng == 0),
                    stop=(ng == NGRP - 1),
                )

        # final: evict the [1,512] accumulators side by side on partition 0
        # (engines can only write at partition base 0), single DMA out
        rowout = rowpool.tile([1, S_PER * 512], F32, tag="rowout", name="rowout")
        for s in range(S_PER):
            nc.scalar.copy(rowout[:, s * 512 : (s + 1) * 512], pes[s][:, :])
        nc.sync.dma_start(out=outp[:, :], in_=rowout[:, :])

    return nc


_NC_CACHE = None


def _get_nc() -> bass.Bass:
    global _NC_CACHE
    if _NC_CACHE is None:
        _NC_CACHE = _build_nc()
    return _NC_CACHE


def kernel(x: np.ndarray, scale_weights: np.ndarray, _trace: bool = False) -> np.ndarray:
    global LAST_RESULTS
    x = np.asarray(x, dtype=np.float32)
    scale_weights = np.asarray(scale_weights, dtype=np.float32)
    assert x.shape == (P, NT) and scale_weights.shape == (S_TOTAL,)

    # host prep: zero-pad, transpose to time-major blocked layout
    xpad = np.zeros((NBLK * P, P), dtype=np.float32)
    xpad[P : P + NT, :] = x.T
    # xb2[a, A*128 + b] = xpad[A*128 + a, b]
    xb2 = np.ascontiguousarray(
        xpad.reshape(NBLK, P, P).transpose(1, 0, 2).reshape(P, NBLK * P)
    )

    G = _toeplitz_weights()  # [32, 3, 128, 128]
    # combined per-core input: [weights | x | ones], bf16 for the 1-col/cycle
    # matmul stream; core c handles scales [4c, 4c+4)
    import ml_dtypes

    bf16 = ml_dtypes.bfloat16
    ones = np.ones((P, 1), dtype=np.float32)
    xgs = []
    for c in range(N_CORES):
        Gc = G[c * S_PER : (c + 1) * S_PER].reshape(S_PER * 3, P, P)
        gw2 = Gc.transpose(1, 0, 2).reshape(P, GCOLS)
        xgs.append(
            np.ascontiguousarray(
                np.concatenate([gw2, xb2, ones], axis=1).astype(bf16)
            )
        )

    nc = _get_nc()
    in_maps = [{"xg": xgs[c]} for c in range(N_CORES)]
    res = run_bass_kernel_spmd(nc, in_maps, list(range(N_CORES)), trace=_trace)
    LAST_RESULTS = res

    # gather + unshard: [8 cores][1, 4 scales * (4 Bsub * 128 b)] -> [128, 32]
    esum = np.concatenate(
        [res.results[c]["outp"].reshape(S_PER, 512) for c in range(N_CORES)],
        axis=0,
    )  # [32, 512]
    esum = esum.reshape(S_TOTAL, 4, P).sum(axis=1)  # fold Bsub -> [32, 128]
    energy = esum.T / np.float32(NT)

    w = scale_weights.astype(np.float64)
    e = np.exp(w - w.max())
    sm = (e / e.sum()).astype(np.float32)
    return (energy * sm[None, :]).astype(np.float32)


if __name__ == "__main__":
    rng = np.random.default_rng(0)
    x = rng.standard_normal((P, NT), dtype=np.float32)
    sw = rng.standard_normal(S_TOTAL, dtype=np.float32)
    out = kernel(x, sw)
    print("kernel output shape:", out.shape, out.dtype)



# revision 13
# speedup vs baseline: 1.3275x; 1.3275x over previous
"""Trainium2 Bass kernel for ContinuousWaveletLayer (CWT energy), v2.

Reference computation:
  bank = Morlet wavelet bank [32 scales, Lmax=256] (static)
  coef[b,s,t] = 'same' conv of x[b,:] (len 8192) with bank[s,:]
  out[b,s]    = mean_t(coef^2) * softmax(scale_weights)[s]

v2 strategy (vs bf16 3-matmul Toeplitz baseline):
  * fp8(e4m3) weights+x with DoubleRow matmuls (K=256 per pass).
  * Scale s has support width 8s centered at 127.5 in the 256 buffer, so
    scales 1..16 fit a single K=256 window IF the x window is shifted by
    +64: one DoubleRow MM per (scale, 4-block group) instead of three
    bf16 MMs.  Large scales (17..32) take a DoubleRow pass + one plain
    fp8 K=128 pass.  Each core gets 2 small + 2 large scales (balanced).
  * The fp8 quantization error in the energies is dominated by a
    deterministic per-scale ||w_q||^2 / ||w||^2 factor (plus a per-row
    ||x_q||^2 factor) which the host divides out exactly.
  * Input DMA is split into a weights chunk + 8 x-slab chunks laid out
    chunk-major so each conv MM's rhs sits inside exactly one chunk
    (the walrus build allows one sync wait per instruction).  A tiny
    PE "guard" matmul per chunk carries the DMA wait; warmup matmuls on
    the weights chunk run during the DMA to trip the HAM clock gate.
"""

import sys
from contextlib import ExitStack

import numpy as np

sys.path.insert(0, "/opt/trn_rl_repo")

import concourse.bass as bass
import concourse.mybir as mybir
from concourse import tile
from concourse.bass_utils import run_bass_kernel_spmd
from concourse.vector_clock import ScopedClock


def _drain_and_barrier_single_wait(self, tick_clock, wait_clock):
    """TileContext._drain_and_barrier, but the kernel-tail drain's
    global-clock waits are spread over a chain of single-wait drains —
    the walrus build here allows only one sync wait per instruction."""
    drain_inst = self.nc.sync.drain()
    wait_clock.add_sem_waits(
        drain_inst.ins, ScopedClock({None: tick_clock.global_clock})
    )
    si = drain_inst.ins.sync_info
    waits = list(si.on_wait)
    if len(waits) > 1:
        si.on_wait = [waits[0]]
        sems = {h.name: h for h in self.sems.allocated().values()}
        for w in waits[1:]:
            d2 = self.nc.sync.drain()
            d2.wait_op(sems[w.ant_name], w.wait_value, "sem-ge")
    self.nc.all_engine_barrier()
    assert self.sems is not None
    popped = self.nc._tile_sem_poison_stack.pop()
    assert popped is self._sem_poison
    self.nc.clear_and_free_semaphores(list(self.sems.allocated().values()))
    self.nc.all_engine_barrier()


tile.TileContext._drain_and_barrier = _drain_and_barrier_single_wait

N_CORES = 8
S_TOTAL = 32          # number of scales
S_PER = 4             # scales per core: 2 small (1..16) + 2 large (17..32)
P = 128               # partition / block size
NT = 8192             # time samples
LMAX = 256            # padded kernel length
NBLK_U = 66           # unshifted input blocks: (128 + 8192 + 128) / 128
NBLK_S = 65           # +64-shifted input blocks
NGRP = 16             # groups of 4 output blocks (N=512 matmuls)
NCHUNK = 8            # x DMA chunks (2 groups each)
F32 = mybir.dt.float32
BF16 = mybir.dt.bfloat16
FP8 = mybir.dt.float8e4

WCOL = 2 * 256 + 2 * 384            # 1280 weight cols (2 small + 2 large)
# Only 8 DMAHW semaphore lanes exist; with 2 output DMAs that leaves 6
# input chunks (weights ride in chunk 0).  Chunk c covers CHUNK_GROUPS[c]
# output groups; its slab carries 2 (xu) / 1 (xs) lookahead blocks.
CHUNK_GROUPS = [3, 3, 3, 3, 2, 2]
CHUNK_FIRST_GROUP = [0, 3, 6, 9, 12, 14]
CHUNK_XU_BLK = [4 * g + 2 for g in CHUNK_GROUPS]
CHUNK_XS_BLK = [4 * g + 1 for g in CHUNK_GROUPS]
CHUNK_BASE = []
_col = WCOL
for _g, _u, _s in zip(CHUNK_GROUPS, CHUNK_XU_BLK, CHUNK_XS_BLK):
    CHUNK_BASE.append(_col)
    _col += (_u + _s) * P
NCOL = _col
NCHUNK = len(CHUNK_GROUPS)

GROUP_CHUNK = []                    # ng -> (chunk, within-chunk group idx)
for _c, (_f, _g) in enumerate(zip(CHUNK_FIRST_GROUP, CHUNK_GROUPS)):
    for _r in range(_g):
        GROUP_CHUNK.append((_c, _r))

LAST_RESULTS = None   # BassKernelResults of the most recent run (for test.py)


def _morlet_kernel_bank(n_scales: int, n: int) -> np.ndarray:
    Lmax = min(8 * n_scales, n)
    bank = np.zeros((n_scales, Lmax), dtype=np.float32)
    for i, s in enumerate(range(1, n_scales + 1)):
        L = min(8 * s, n)
        t = np.linspace(-4.0 * s, 4.0 * s, L)
        w = np.exp(-t**2 / (2.0 * s**2)) * np.cos(5.0 * t / s)
        w = w / np.sqrt(s)
        off = (Lmax - 1) // 2 - (L - 1) // 2
        bank[i, off : off + L] = w.astype(np.float32)
    return bank


def _core_scales(c: int) -> list[int]:
    """0-based scale indices handled by core c: 2 small then 2 large."""
    return [2 * c, 2 * c + 1, 16 + 2 * c, 16 + 2 * c + 1]


def _toeplitz_cols(gq: np.ndarray) -> np.ndarray:
    """Per-core weight columns [128, WCOL] from quantized reversed bank gq.

    Small scale i at cols [256i, 256i+256):  (p, j*128+to) = g'[64+128j+p-to]
    Large scale i at cols [512+384i, ...):   G2 (p, j*128+to) = g'[128j+p-to]
                                             G3 (p, 256+to)   = g'[256+p-to]
    """
    p = np.arange(P)[:, None]
    to = np.arange(P)[None, :]

    def gslice(row, d):
        v = np.zeros(d.shape, dtype=np.float32)
        ok = (d >= 0) & (d < LMAX)
        v[ok] = row[np.clip(d, 0, LMAX - 1)][ok]
        return v

    w = np.zeros((P, WCOL), dtype=np.float32)
    for i in range(2):          # small scales
        row = gq[i]
        base = 256 * i
        for j in range(2):
            w[:, base + 128 * j : base + 128 * (j + 1)] = gslice(
                row, 64 + 128 * j + p - to
            )
    for i in range(2):          # large scales
        row = gq[2 + i]
        base = 512 + 384 * i
        for j in range(2):
            w[:, base + 128 * j : base + 128 * (j + 1)] = gslice(
                row, 128 * j + p - to
            )
        w[:, base + 256 : base + 384] = gslice(row, 256 + p - to)
    return w


def _build_nc() -> bass.Bass:
    nc = bass.Bass()
    xg = nc.dram_tensor("xg", [P, NCOL], FP8, kind="ExternalInput")
    outp = nc.dram_tensor("outp", [1, S_PER * 512], F32, kind="ExternalOutput")

    with tile.TileContext(nc) as tc, ExitStack() as ctx:
        xpool = ctx.enter_context(tc.tile_pool(name="x", bufs=1))
        sqpool = ctx.enter_context(tc.tile_pool(name="sq", bufs=NGRP * S_PER))
        cppool = ctx.enter_context(tc.tile_pool(name="cp", bufs=4))
        rowpool = ctx.enter_context(tc.tile_pool(name="row", bufs=4))
        onepool = ctx.enter_context(tc.tile_pool(name="one", bufs=1))
        pspool = ctx.enter_context(tc.tile_pool(name="ps", bufs=4, space="PSUM"))
        psepool = ctx.enter_context(tc.tile_pool(name="pse", bufs=1, space="PSUM"))

        xgsb = xpool.tile([P, NCOL], FP8)
        # chunked input DMA in consumption order (6 chunks; chunk 0 also
        # carries the weight columns). SP queue, one DMAHW lane each.
        for c in range(NCHUNK):
            lo = 0 if c == 0 else CHUNK_BASE[c]
            hi = CHUNK_BASE[c] + (CHUNK_XU_BLK[c] + CHUNK_XS_BLK[c]) * P
            nc.sync.dma_start(out=xgsb[:, lo:hi], in_=xg[:, lo:hi])

        onesb = onepool.tile([P, 1], BF16, tag="ones", name="ones")
        nc.vector.memset(onesb[:, :], 1.0)

        # warmup matmuls on the weights chunk: trip the HAM clock gate
        # while the first x slab is still in flight (no consumers)
        for wi in range(4):
            wt = pspool.tile([P, 512], F32, tag="conv")
            nc.tensor.matmul(
                wt[:, :], xgsb[:, 0:P], xgsb[:, 0:512], start=True, stop=True
            )

        # per-scale PSUM energy accumulators [1, (Bsub, b)]
        pes = [
            psepool.tile([1, 512], F32, tag=f"pe{s}", name=f"pe{s}")
            for s in range(S_PER)
        ]

        DR = mybir.MatmulPerfMode.DoubleRow
        ORDER = [0, 2, 1, 3]   # small, large, small, large
        for ng in range(NGRP):
            c, r = GROUP_CHUNK[ng]
            ch_base = CHUNK_BASE[c]
            xu_base = ch_base + 4 * r * P
            xs_base = ch_base + CHUNK_XU_BLK[c] * P + 4 * r * P
            if r == 0:
                # guard matmul: carries the chunk-c DMA wait so the real
                # conv matmuls below never need a second sem wait
                gt = pspool.tile([P, 512], F32, tag="conv")
                nc.tensor.matmul(
                    gt[:, :64],
                    xgsb[:, ch_base : ch_base + P],
                    xgsb[:, ch_base : ch_base + 64],
                    start=True,
                    stop=True,
                )
            for pos, si in enumerate(ORDER):
                i = ng * S_PER + pos
                pt = pspool.tile([P, 512], F32, tag="conv")
                if si < 2:      # small scale: single DoubleRow pass
                    lhsT = xgsb[:, 256 * si : 256 * si + 256].rearrange(
                        "p (j t) -> p j t", j=2
                    )
                    nc.tensor.matmul(
                        pt[:, :], lhsT, _xwin(xgsb, xs_base), start=True,
                        stop=True, perf_mode=DR,
                    )
                else:           # large scale: DoubleRow + plain K=128 pass
                    base_w = 512 + 384 * (si - 2)
                    lhsT2 = xgsb[:, base_w : base_w + 256].rearrange(
                        "p (j t) -> p j t", j=2
                    )
                    nc.tensor.matmul(
                        pt[:, :], lhsT2, _xwin(xgsb, xu_base), start=True,
                        stop=False, perf_mode=DR,
                    )
                    lhsT3 = xgsb[:, base_w + 256 : base_w + 384]
                    rhs3 = xgsb[:, xu_base + 2 * P : xu_base + 6 * P]
                    nc.tensor.matmul(
                        pt[:, :], lhsT3, rhs3, start=False, stop=True,
                    )
                sq = sqpool.tile([P, 512], BF16)
                if i % 2 == 0:
                    # ACT path: square+cast straight out of PSUM
                    nc.scalar.square(sq[:, :], pt[:, :])
                else:
                    # DVE path: bf16 copy out of PSUM, then bf16 square
                    cp = cppool.tile([P, 512], BF16)
                    nc.vector.tensor_copy(cp[:, :], pt[:, :])
                    nc.vector.tensor_mul(sq[:, :], cp[:, :], cp[:, :])
                nc.tensor.matmul(
                    pes[si][:, :],
                    onesb[:, :],
                    sq[:, :],
                    start=(ng == 0),
                    stop=(ng == NGRP - 1),
                )

        # final: evict the [1,512] accumulators side by side on partition 0.
        # ACT evicts scales {0,2} into the low half, DVE evicts {1,3} into
        # the high half; each half goes out via its own DMA so every DMA
        # carries exactly one engine sem wait (host un-permutes).
        rowout = rowpool.tile([1, S_PER * 512], F32, tag="rowout", name="rowout")
        nc.scalar.copy(rowout[:, 0:512], pes[0][:, :])
        nc.scalar.copy(rowout[:, 512:1024], pes[2][:, :])
        nc.vector.tensor_copy(rowout[:, 1024:1536], pes[1][:, :])
        nc.vector.tensor_copy(rowout[:, 1536:2048], pes[3][:, :])
        # Activation-engine DMA queue: the SP queue's ring credits are
        # used up by the 9 input chunks (a 10th SP DMA would carry a
        # ring-credit wait on top of its data dep = 2 waits)
        nc.scalar.dma_start(out=outp[:, 0:1024], in_=rowout[:, 0:1024])
        nc.scalar.dma_start(out=outp[:, 1024:2048], in_=rowout[:, 1024:2048])

    return nc


def _xwin(xgsb, base):
    """rhs AP [128, j=2 (stride 128 cols, overlapping), 512] at col base."""
    sl = xgsb[:, base : base + 5 * P]
    return bass.AP(sl.tensor, sl.offset, [list(sl.ap[0]), [P, 2], [1, 512]])


_NC_CACHE = None


def _get_nc() -> bass.Bass:
    global _NC_CACHE
    if _NC_CACHE is None:
        _NC_CACHE = _build_nc()
    return _NC_CACHE


def kernel(x: np.ndarray, scale_weights: np.ndarray, _trace: bool = False) -> np.ndarray:
    global LAST_RESULTS
    import ml_dtypes

    e4 = ml_dtypes.float8_e4m3fn
    x = np.asarray(x, dtype=np.float32)
    scale_weights = np.asarray(scale_weights, dtype=np.float32)
    assert x.shape == (P, NT) and scale_weights.shape == (S_TOTAL,)

    bank = _morlet_kernel_bank(S_TOTAL, NT)          # [32, 256] fp32
    gq = bank[:, ::-1].astype(e4).astype(np.float32)  # quantized g' rows

    xq8 = x.T.astype(e4)                              # [NT, P] fp8
    xq = xq8.astype(np.float32)

    # time-major blocked layouts (fp8 bytes)
    xpad = np.zeros((NBLK_U * P, P), dtype=e4)
    xpad[P : P + NT, :] = xq8
    xb_u = xpad.reshape(NBLK_U, P, P).transpose(1, 0, 2).reshape(P, NBLK_U * P)
    xsh = xpad[64 : 64 + NBLK_S * P, :]
    xb_s = np.ascontiguousarray(xsh).reshape(NBLK_S, P, P).transpose(1, 0, 2).reshape(
        P, NBLK_S * P
    )

    xgs = []
    for c in range(N_CORES):
        scales = _core_scales(c)
        w = _toeplitz_cols(gq[scales]).astype(e4)     # [128, WCOL]
        buf = np.empty((P, NCOL), dtype=e4)
        buf[:, :WCOL] = w
        for ch in range(NCHUNK):
            lo = CHUNK_BASE[ch]
            u0 = 4 * CHUNK_FIRST_GROUP[ch] * P
            nu, ns = CHUNK_XU_BLK[ch] * P, CHUNK_XS_BLK[ch] * P
            buf[:, lo : lo + nu] = xb_u[:, u0 : u0 + nu]
            buf[:, lo + nu : lo + nu + ns] = xb_s[:, u0 : u0 + ns]
        xgs.append(buf)

    nc = _get_nc()
    in_maps = [{"xg": xgs[c]} for c in range(N_CORES)]
    res = run_bass_kernel_spmd(nc, in_maps, list(range(N_CORES)), trace=_trace)
    LAST_RESULTS = res

    # gather + unshard: core c covers scale ids [2c, 2c+1, 16+2c, 17+2c]
    esum = np.zeros((S_TOTAL, P), dtype=np.float64)
    for c in range(N_CORES):
        vals = res.results[c]["outp"].reshape(S_PER, 4, P).sum(axis=1)
        sc = _core_scales(c)
        # device row order is [si=0, 2, 1, 3] (ACT low half, DVE high half)
        for row, si in enumerate([0, 2, 1, 3]):
            esum[sc[si]] = vals[row]
    energy = esum.T / np.float64(NT)                  # [128 b, 32 s]

    # exact correction of the deterministic fp8 norm bias
    w2 = (bank.astype(np.float64) ** 2).sum(1)        # [32]
    wq2 = (gq.astype(np.float64) ** 2).sum(1)
    mx2 = (x.astype(np.float64) ** 2).mean(1)         # [128]
    mxq2 = (xq.T.astype(np.float64) ** 2).mean(1)
    energy = energy * (mx2[:, None] * w2[None, :]) / (mxq2[:, None] * wq2[None, :])

    w = scale_weights.astype(np.float64)
    e = np.exp(w - w.max())
    sm = e / e.sum()
    return (energy * sm[None, :]).astype(np.float32)


if __name__ == "__main__":
    rng = np.random.default_rng(0)
    x = rng.standard_normal((P, NT), dtype=np.float32)
    sw = rng.standard_normal(S_TOTAL, dtype=np.float32)
    out = kernel(x, sw)
    print("kernel output shape:", out.shape, out.dtype)


# revision 18
# speedup vs baseline: 1.5791x; 1.1895x over previous
"""Trainium2 Bass kernel for ContinuousWaveletLayer (CWT energy), v2.

Reference computation:
  bank = Morlet wavelet bank [32 scales, Lmax=256] (static)
  coef[b,s,t] = 'same' conv of x[b,:] (len 8192) with bank[s,:]
  out[b,s]    = mean_t(coef^2) * softmax(scale_weights)[s]

v2 strategy (vs bf16 3-matmul Toeplitz baseline):
  * fp8(e4m3) weights+x with DoubleRow matmuls (K=256 per pass).
  * Scale s has support width 8s centered at 127.5 in the 256 buffer, so
    scales 1..16 fit a single K=256 window IF the x window is shifted by
    +64: one DoubleRow MM per (scale, 4-block group) instead of three
    bf16 MMs.  Large scales (17..32) take a DoubleRow pass + one plain
    fp8 K=128 pass.  Each core gets 2 small + 2 large scales (balanced).
  * The fp8 quantization error in the energies is dominated by a
    deterministic per-scale ||w_q||^2 / ||w||^2 factor (plus a per-row
    ||x_q||^2 factor) which the host divides out exactly.
  * Input DMA is split into a weights chunk + 8 x-slab chunks laid out
    chunk-major so each conv MM's rhs sits inside exactly one chunk
    (the walrus build allows one sync wait per instruction).  A tiny
    PE "guard" matmul per chunk carries the DMA wait; warmup matmuls on
    the weights chunk run during the DMA to trip the HAM clock gate.
"""

import sys
from contextlib import ExitStack

import numpy as np

sys.path.insert(0, "/opt/trn_rl_repo")

import concourse.bass as bass
import concourse.mybir as mybir
from concourse import tile
from concourse.bass_utils import run_bass_kernel_spmd
from concourse.vector_clock import ScopedClock


def _drain_and_barrier_single_wait(self, tick_clock, wait_clock):
    """TileContext._drain_and_barrier, but the kernel-tail drain's
    global-clock waits are spread over a chain of single-wait drains —
    the walrus build here allows only one sync wait per instruction."""
    drain_inst = self.nc.sync.drain()
    wait_clock.add_sem_waits(
        drain_inst.ins, ScopedClock({None: tick_clock.global_clock})
    )
    si = drain_inst.ins.sync_info
    waits = list(si.on_wait)
    if len(waits) > 1:
        si.on_wait = [waits[0]]
        sems = {h.name: h for h in self.sems.allocated().values()}
        for w in waits[1:]:
            d2 = self.nc.sync.drain()
            d2.wait_op(sems[w.ant_name], w.wait_value, "sem-ge")
    self.nc.all_engine_barrier()
    assert self.sems is not None
    popped = self.nc._tile_sem_poison_stack.pop()
    assert popped is self._sem_poison
    self.nc.clear_and_free_semaphores(list(self.sems.allocated().values()))
    self.nc.all_engine_barrier()


tile.TileContext._drain_and_barrier = _drain_and_barrier_single_wait

N_CORES = 8
S_TOTAL = 32          # number of scales
S_PER = 4             # scales per core: 2 small (1..16) + 2 large (17..32)
P = 128               # partition / block size
NT = 8192             # time samples
LMAX = 256            # padded kernel length
NBLK_U = 66           # unshifted input blocks: (128 + 8192 + 128) / 128
NBLK_S = 65           # +64-shifted input blocks
NGRP = 16             # groups of 4 output blocks (N=512 matmuls)
NCHUNK = 8            # x DMA chunks (2 groups each)
F32 = mybir.dt.float32
BF16 = mybir.dt.bfloat16
FP8 = mybir.dt.float8e4

WCOL = 2 * 256 + 2 * 384            # 1280 weight cols (2 small + 2 large)
# Only 8 DMAHW semaphore lanes exist; with 2 output DMAs that leaves 6
# input chunks (weights ride in chunk 0).  Chunk c covers CHUNK_GROUPS[c]
# output groups; its slab carries 2 (xu) / 1 (xs) lookahead blocks.
CHUNK_GROUPS = [2, 4, 4, 3, 3]
CHUNK_FIRST_GROUP = [0, 2, 6, 10, 13]
CHUNK_XU_BLK = [4 * g + 2 for g in CHUNK_GROUPS]
CHUNK_XS_BLK = [4 * g + 1 for g in CHUNK_GROUPS]
CHUNK_BASE = []
_col = WCOL
for _g, _u, _s in zip(CHUNK_GROUPS, CHUNK_XU_BLK, CHUNK_XS_BLK):
    CHUNK_BASE.append(_col)
    _col += (_u + _s) * P
NCOL = _col
NCHUNK = len(CHUNK_GROUPS)

GROUP_CHUNK = []                    # ng -> (chunk, within-chunk group idx)
for _c, (_f, _g) in enumerate(zip(CHUNK_FIRST_GROUP, CHUNK_GROUPS)):
    for _r in range(_g):
        GROUP_CHUNK.append((_c, _r))

LAST_RESULTS = None   # BassKernelResults of the most recent run (for test.py)


def _morlet_kernel_bank(n_scales: int, n: int) -> np.ndarray:
    Lmax = min(8 * n_scales, n)
    bank = np.zeros((n_scales, Lmax), dtype=np.float32)
    for i, s in enumerate(range(1, n_scales + 1)):
        L = min(8 * s, n)
        t = np.linspace(-4.0 * s, 4.0 * s, L)
        w = np.exp(-t**2 / (2.0 * s**2)) * np.cos(5.0 * t / s)
        w = w / np.sqrt(s)
        off = (Lmax - 1) // 2 - (L - 1) // 2
        bank[i, off : off + L] = w.astype(np.float32)
    return bank


def _core_scales(c: int) -> list[int]:
    """0-based scale indices handled by core c: 2 small then 2 large."""
    return [2 * c, 2 * c + 1, 16 + 2 * c, 16 + 2 * c + 1]


def _toeplitz_cols(gq: np.ndarray) -> np.ndarray:
    """Per-core weight columns [128, WCOL] from quantized reversed bank gq.

    Small scale i at cols [256i, 256i+256):  (p, j*128+to) = g'[64+128j+p-to]
    Large scale i at cols [512+384i, ...):   G2 (p, j*128+to) = g'[128j+p-to]
                                             G3 (p, 256+to)   = g'[256+p-to]
    """
    p = np.arange(P)[:, None]
    to = np.arange(P)[None, :]

    def gslice(row, d):
        v = np.zeros(d.shape, dtype=np.float32)
        ok = (d >= 0) & (d < LMAX)
        v[ok] = row[np.clip(d, 0, LMAX - 1)][ok]
        return v

    w = np.zeros((P, WCOL), dtype=np.float32)
    for i in range(2):          # small scales
        row = gq[i]
        base = 256 * i
        for j in range(2):
            w[:, base + 128 * j : base + 128 * (j + 1)] = gslice(
                row, 64 + 128 * j + p - to
            )
    for i in range(2):          # large scales
        row = gq[2 + i]
        base = 512 + 384 * i
        for j in range(2):
            w[:, base + 128 * j : base + 128 * (j + 1)] = gslice(
                row, 128 * j + p - to
            )
        w[:, base + 256 : base + 384] = gslice(row, 256 + p - to)
    return w


def _build_nc() -> bass.Bass:
    nc = bass.Bass()
    xg = nc.dram_tensor("xg", [P, NCOL], FP8, kind="ExternalInput")
    outp = nc.dram_tensor("outp", [1, S_PER * 512], F32, kind="ExternalOutput")

    with tile.TileContext(nc) as tc, ExitStack() as ctx:
        xpool = ctx.enter_context(tc.tile_pool(name="x", bufs=1))
        sqpool = ctx.enter_context(tc.tile_pool(name="sq", bufs=1))
        cppool = ctx.enter_context(tc.tile_pool(name="cp", bufs=4))
        rowpool = ctx.enter_context(tc.tile_pool(name="row", bufs=4))
        onepool = ctx.enter_context(tc.tile_pool(name="one", bufs=1))
        pspool = ctx.enter_context(tc.tile_pool(name="ps", bufs=4, space="PSUM"))
        psepool = ctx.enter_context(tc.tile_pool(name="pse", bufs=1, space="PSUM"))

        xgsb = xpool.tile([P, NCOL], FP8)
        # chunked input DMA in consumption order: weights first (small,
        # unblocks the HAM warmup matmuls early), then 5 x chunks.
        # 6 input + 2 output DMAs = all 8 DMAHW lanes, no recycling.
        nc.sync.dma_start(out=xgsb[:, :WCOL], in_=xg[:, :WCOL])
        for c in range(NCHUNK):
            lo = CHUNK_BASE[c]
            hi = lo + (CHUNK_XU_BLK[c] + CHUNK_XS_BLK[c]) * P
            nc.sync.dma_start(out=xgsb[:, lo:hi], in_=xg[:, lo:hi])

        onesb = onepool.tile([P, 32], FP8, tag="ones", name="ones")
        nc.vector.memset(onesb[:, :], 1.0)

        # warmup matmuls on the weights chunk: trip the HAM clock gate
        # while the first x slab is still in flight (no consumers)
        for wi in range(4):
            wt = pspool.tile([P, 512], F32, tag="conv")
            nc.tensor.matmul(
                wt[:, :], xgsb[:, 0:P], xgsb[:, 0:512], start=True, stop=True
            )

        # per-scale PSUM energy accumulators [1, (Bsub, b)]
        pes = [
            psepool.tile([1, 512], F32, tag=f"pe{s}", name=f"pe{s}")
            for s in range(S_PER)
        ]

        DR = mybir.MatmulPerfMode.DoubleRow
        ORDER = [0, 2, 1, 3]   # small, large, small, large
        sqtiles = {}           # (si, pair) -> fp8 pair tile [128, 1024]
        for ng in range(NGRP):
            c, r = GROUP_CHUNK[ng]
            ch_base = CHUNK_BASE[c]
            xu_base = ch_base + 4 * r * P
            xs_base = ch_base + CHUNK_XU_BLK[c] * P + 4 * r * P
            if r == 0:
                # guard matmul: carries the chunk-c DMA wait so the real
                # conv matmuls below never need a second sem wait
                gt = pspool.tile([P, 512], F32, tag="conv")
                nc.tensor.matmul(
                    gt[:, :64],
                    xgsb[:, ch_base : ch_base + P],
                    xgsb[:, ch_base : ch_base + 64],
                    start=True,
                    stop=True,
                )
            for pos, si in enumerate(ORDER):
                i = ng * S_PER + pos
                pt = pspool.tile([P, 512], F32, tag="conv")
                if si < 2:      # small scale: single DoubleRow pass
                    lhsT = xgsb[:, 256 * si : 256 * si + 256].rearrange(
                        "p (j t) -> p j t", j=2
                    )
                    nc.tensor.matmul(
                        pt[:, :], lhsT, _xwin(xgsb, xs_base), start=True,
                        stop=True, perf_mode=DR,
                    )
                else:           # large scale: DoubleRow + plain K=128 pass
                    base_w = 512 + 384 * (si - 2)
                    lhsT2 = xgsb[:, base_w : base_w + 256].rearrange(
                        "p (j t) -> p j t", j=2
                    )
                    nc.tensor.matmul(
                        pt[:, :], lhsT2, _xwin(xgsb, xu_base), start=True,
                        stop=False, perf_mode=DR,
                    )
                    lhsT3 = xgsb[:, base_w + 256 : base_w + 384]
                    rhs3 = xgsb[:, xu_base + 2 * P : xu_base + 6 * P]
                    nc.tensor.matmul(
                        pt[:, :], lhsT3, rhs3, start=False, stop=True,
                    )
                # squares land in fp8 pair tiles [128, (half, 512)]; one
                # DoubleRow reduce per pair halves the reduce matmul count.
                # Both halves of a pair use the SAME engine (single wait).
                pair = ng // 2
                half = ng % 2
                pidx = pair * S_PER + si
                if half == 0:
                    sqtiles[(si, pair)] = sqpool.tile(
                        [P, 1024], FP8, name=f"sq{si}_{pair}"
                    )
                sq = sqtiles[(si, pair)]
                dst = sq[:, half * 512 : half * 512 + 512]
                if pidx % 8 not in (1, 4, 7):
                    # ACT path: square+cast straight out of PSUM (20/32)
                    nc.scalar.square(dst, pt[:, :])
                else:
                    # DVE path: bf16 copy out of PSUM, then square (12/32)
                    cp = cppool.tile([P, 512], BF16)
                    nc.vector.tensor_copy(cp[:, :], pt[:, :])
                    nc.vector.tensor_mul(dst, cp[:, :], cp[:, :])
                if half == 1:
                    ones_dr = bass.AP(
                        onesb.tensor, onesb[:, :].offset,
                        [list(onesb[:, :].ap[0]), [16, 2], [1, 1]],
                    )
                    nc.tensor.matmul(
                        pes[si][:, :],
                        ones_dr,
                        sq[:, :].rearrange("p (j n) -> p j n", j=2),
                        start=(ng == 1),
                        stop=(ng == NGRP - 1),
                        perf_mode=DR,
                    )

        # final: evict the [1,512] accumulators side by side on partition 0.
        # ACT evicts scales {0,2} into the low half, DVE evicts {1,3} into
        # the high half; each half goes out via its own DMA so every DMA
        # carries exactly one engine sem wait (host un-permutes).
        rowout = rowpool.tile([1, S_PER * 512], F32, tag="rowout", name="rowout")
        nc.scalar.copy(rowout[:, 0:512], pes[0][:, :])
        nc.scalar.copy(rowout[:, 512:1024], pes[2][:, :])
        nc.vector.tensor_copy(rowout[:, 1024:1536], pes[1][:, :])
        nc.vector.tensor_copy(rowout[:, 1536:2048], pes[3][:, :])
        # Activation-engine DMA queue: the SP queue's ring credits are
        # used up by the 9 input chunks (a 10th SP DMA would carry a
        # ring-credit wait on top of its data dep = 2 waits)
        nc.scalar.dma_start(out=outp[:, 0:1024], in_=rowout[:, 0:1024])
        nc.scalar.dma_start(out=outp[:, 1024:2048], in_=rowout[:, 1024:2048])

    return nc


def _xwin(xgsb, base):
    """rhs AP [128, j=2 (stride 128 cols, overlapping), 512] at col base."""
    sl = xgsb[:, base : base + 5 * P]
    return bass.AP(sl.tensor, sl.offset, [list(sl.ap[0]), [P, 2], [1, 512]])


_NC_CACHE = None


def _get_nc() -> bass.Bass:
    global _NC_CACHE
    if _NC_CACHE is None:
        _NC_CACHE = _build_nc()
    return _NC_CACHE


def kernel(x: np.ndarray, scale_weights: np.ndarray, _trace: bool = False) -> np.ndarray:
    global LAST_RESULTS
    import ml_dtypes

    e4 = ml_dtypes.float8_e4m3fn
    x = np.asarray(x, dtype=np.float32)
    scale_weights = np.asarray(scale_weights, dtype=np.float32)
    assert x.shape == (P, NT) and scale_weights.shape == (S_TOTAL,)

    bank = _morlet_kernel_bank(S_TOTAL, NT)          # [32, 256] fp32
    gq = bank[:, ::-1].astype(e4).astype(np.float32)  # quantized g' rows

    xq8 = x.T.astype(e4)                              # [NT, P] fp8
    xq = xq8.astype(np.float32)

    # time-major blocked layouts (fp8 bytes)
    xpad = np.zeros((NBLK_U * P, P), dtype=e4)
    xpad[P : P + NT, :] = xq8
    xb_u = xpad.reshape(NBLK_U, P, P).transpose(1, 0, 2).reshape(P, NBLK_U * P)
    xsh = xpad[64 : 64 + NBLK_S * P, :]
    xb_s = np.ascontiguousarray(xsh).reshape(NBLK_S, P, P).transpose(1, 0, 2).reshape(
        P, NBLK_S * P
    )

    xgs = []
    for c in range(N_CORES):
        scales = _core_scales(c)
        w = _toeplitz_cols(gq[scales]).astype(e4)     # [128, WCOL]
        buf = np.empty((P, NCOL), dtype=e4)
        buf[:, :WCOL] = w
        for ch in range(NCHUNK):
            lo = CHUNK_BASE[ch]
            u0 = 4 * CHUNK_FIRST_GROUP[ch] * P
            nu, ns = CHUNK_XU_BLK[ch] * P, CHUNK_XS_BLK[ch] * P
            buf[:, lo : lo + nu] = xb_u[:, u0 : u0 + nu]
            buf[:, lo + nu : lo + nu + ns] = xb_s[:, u0 : u0 + ns]
        xgs.append(buf)

    nc = _get_nc()
    in_maps = [{"xg": xgs[c]} for c in range(N_CORES)]
    res = run_bass_kernel_spmd(nc, in_maps, list(range(N_CORES)), trace=_trace)
    LAST_RESULTS = res

    # gather + unshard: core c covers scale ids [2c, 2c+1, 16+2c, 17+2c]
    esum = np.zeros((S_TOTAL, P), dtype=np.float64)
    for c in range(N_CORES):
        vals = res.results[c]["outp"].reshape(S_PER, 4, P).sum(axis=1)
        sc = _core_scales(c)
        # device row order is [si=0, 2, 1, 3] (ACT low half, DVE high half)
        for row, si in enumerate([0, 2, 1, 3]):
            esum[sc[si]] = vals[row]
    energy = esum.T / np.float64(NT)                  # [128 b, 32 s]

    # exact correction of the deterministic fp8 norm bias
    w2 = (bank.astype(np.float64) ** 2).sum(1)        # [32]
    wq2 = (gq.astype(np.float64) ** 2).sum(1)
    mx2 = (x.astype(np.float64) ** 2).mean(1)         # [128]
    mxq2 = (xq.T.astype(np.float64) ** 2).mean(1)
    energy = energy * (mx2[:, None] * w2[None, :]) / (mxq2[:, None] * wq2[None, :])

    w = scale_weights.astype(np.float64)
    e = np.exp(w - w.max())
    sm = e / e.sum()
    return (energy * sm[None, :]).astype(np.float32)


if __name__ == "__main__":
    rng = np.random.default_rng(0)
    x = rng.standard_normal((P, NT), dtype=np.float32)
    sw = rng.standard_normal(S_TOTAL, dtype=np.float32)
    out = kernel(x, sw)
    print("kernel output shape:", out.shape, out.dtype)
